# revision 18
# baseline (speedup 1.0000x reference)
"""Causal self-attention kernel for Trainium2, 8 NeuronCores.

Problem: B=4, T=2048, C=1024, 16 heads, D=64 (fp32).
Sharding: core i handles batch b=i//2 and head-group hg=i%2 (8 heads each).
Each core computes qkv + attention + its partial projection; the host sums
the two head-group partials per batch and adds b_proj.

Per-core dataflow (all matmuls in fp32r: 4x the fp32 PE rate):
  A: x [T,C] loaded natural, PE-transposed to XT [C-chunks of 128, T]
  B: V = x @ Wv natural [T, 512], stored ones-augmented per head ([..|V_h|1|..])
  C: per head-pair g: QT/KT = (Wqk^T @ XT) chunks [128, T];
     scores S^T[k,q] per k-tile via row-packed K=64 matmuls (2 heads at once);
     P^T = exp(S/8) (no max subtraction: |scores| < ~4 for this data);
     causal mask on diagonal strips via gpsimd affine_select;
     O^T/sums accumulated via PV matmuls with ones-augmented V (M=65);
     normalize with reciprocal + K=1 broadcast matmul; odd head partition-
     shifted into the pair-stacked OT via SBUF->SBUF DMA.
  D: out = OT^T @ Wproj natural [T, C] partial, DMA'd out per t-tile.
"""

import numpy as np

N_CORES = 8
T = 2048
C = 1024
HL = 8          # heads per core
D = 64
KC = C // 128   # 8 contraction chunks
NT = T // 128   # 16 t-tiles
NQ = T // 512   # 4 q-tiles
VW = HL * 65    # 520 v-aug cols per t-tile

_CACHE = {}


def _build(phases=('A', 'B', 'QK', 'ATT', 'D')):
    from contextlib import ExitStack
    import concourse.bass as bass
    from concourse import bacc
    import concourse.mybir as mybir
    import concourse.tile as tile
    from concourse.masks import make_identity

    F32 = mybir.dt.float32
    F32R = mybir.dt.float32r
    EXP = mybir.ActivationFunctionType.Exp
    ISGE = mybir.AluOpType.is_ge
    W15 = C + C // 2  # 1536

    nc = bacc.Bacc("TRN2", target_bir_lowering=False, debug=False,
                   num_devices=N_CORES)

    x_d = nc.dram_tensor("x", [T, C], F32, kind="ExternalInput")
    wqkv_d = nc.dram_tensor("w_qkv", [C, W15], F32, kind="ExternalInput")
    wproj_d = nc.dram_tensor("w_proj", [512, C], F32, kind="ExternalInput")
    bqk_d = nc.dram_tensor("b_qk", [128, 8], F32, kind="ExternalInput")
    bv_d = nc.dram_tensor("b_v", [128, 512], F32, kind="ExternalInput")
    ones_d = nc.dram_tensor("ones64", [128, 128], F32, kind="ExternalInput")
    out_d = nc.dram_tensor("out", [T, C], F32, kind="ExternalOutput")

    with tile.TileContext(nc) as tc, ExitStack() as ctx:
        # ---------- persistent pools ----------
        consts = ctx.enter_context(tc.tile_pool(name="consts", bufs=1))
        big = ctx.enter_context(tc.tile_pool(name="big", bufs=1))
        # one psum scope for the whole kernel:
        #   mm  [128,512]x2  - generic matmul outputs (transpose/V/QK/proj)
        #   psS [128,1024]x2 - score strips, shared with the [64,512] bc tiles
        #   o0/o1 [65,512]x1 - PV accumulators
        psmm = ctx.enter_context(tc.tile_pool(name="psmm", bufs=2, space="PSUM"))
        psS = ctx.enter_context(tc.tile_pool(name="psS", bufs=2, space="PSUM"))
        psO = ctx.enter_context(tc.tile_pool(name="psO", bufs=1, space="PSUM"))

        ident = consts.tile([128, 128], F32)
        make_identity(nc, ident[:])
        bqk_sb = consts.tile([128, 8], F32)
        nc.sync.dma_start(out=bqk_sb[:], in_=bqk_d[:])
        bv_sb = consts.tile([128, 512], F32)
        nc.sync.dma_start(out=bv_sb[:], in_=bv_d[:])
        ones_sb = consts.tile([128, 128], F32R)
        nc.sync.dma_start(out=ones_sb[:], in_=ones_d[:].bitcast(F32R))

        XT = big.tile([128, KC * T], F32R)        # 64 KB/part, x transposed
        VA = big.tile([128, NT * VW], F32R)       # 32.5 KB/part, v-aug
        OT = big.tile([128, 4 * T], F32R)         # 32 KB/part, attn out^T

        # ones columns of VA (col 64 of each 65-group, uniform stride 65):
        # one strided DVE copy from the resident ones tile
        nc.vector.tensor_copy(VA[:, 64::65], ones_sb[:])

        # ---------- phases A+B: transpose x; V natural ----------
        if 'A' in phases:
          with (
            tc.tile_pool(name="xnat", bufs=5) as xnat,
            tc.tile_pool(name="wv", bufs=1) as wvp,
          ):
            for it4 in range(NT // 4):
                xts = []
                for j in range(4):
                    it = it4 * 4 + j
                    xt_t = xnat.tile([128, C], F32, tag="xn")
                    eng = (nc.sync, nc.scalar)[it % 2]
                    eng.dma_start(out=xt_t[:], in_=x_d[it * 128:(it + 1) * 128, :])
                    xts.append(xt_t)
                for c in range(KC):
                    pt = psmm.tile([128, 512], F32, tag="mm")
                    for j in range(4):
                        nc.tensor.transpose(
                            pt[:, j * 128:(j + 1) * 128],
                            xts[j][:, c * 128:(c + 1) * 128], ident[:])
                    nc.vector.tensor_copy(
                        XT[:, c * T + it4 * 512: c * T + (it4 + 1) * 512], pt[:])
            if 'B' in phases:
                wv = wvp.tile([128, KC * 512], F32R)
                nc.sync.dma_start(
                    out=wv[:].rearrange("p (k m) -> p k m", k=KC),
                    in_=wqkv_d[:, 1024:1536].rearrange(
                        "(k p) m -> p k m", p=128).bitcast(F32R),
                )
                for it in range(NT):
                    pv = psmm.tile([128, 512], F32, tag="mm")
                    for k in range(KC):
                        nc.tensor.matmul(
                            pv[:],
                            XT[:, k * T + it * 128: k * T + (it + 1) * 128],
                            wv[:, k * 512:(k + 1) * 512],
                            start=(k == 0), stop=(k == KC - 1))
                    va_dst = VA[:, it * VW:(it + 1) * VW].rearrange(
                        "p (h c) -> p h c", h=HL)[:, :, 0:64]
                    nc.vector.tensor_add(
                        va_dst,
                        pv[:].rearrange("p (h c) -> p h c", h=HL),
                        bv_sb[:].rearrange("p (h c) -> p h c", h=HL))

        # ---------- phase C: per-pair qk + attention ----------
        if 'QK' in phases:
          with (
            tc.tile_pool(name="wqk", bufs=1) as wqkp,
            tc.tile_pool(name="qkt", bufs=2) as qktp,
            tc.tile_pool(name="ptile", bufs=3) as ptp,
            tc.tile_pool(name="rsc", bufs=1) as rscp,
          ):
            for g in range(4):
                wqk = wqkp.tile([128, 2 * KC * 128], F32R, tag="wqk")
                for half in (0, 1):
                    nc.sync.dma_start(
                        out=wqk[:, half * KC * 128:(half + 1) * KC * 128]
                            .rearrange("p (k m) -> p k m", k=KC),
                        in_=wqkv_d[:, half * 512 + g * 128: half * 512 + (g + 1) * 128]
                            .rearrange("(k p) m -> p k m", p=128).bitcast(F32R),
                    )
                qkt = qktp.tile([128, 2 * T], F32R, tag="qkt")
                for half in (0, 1):
                    for nt4 in range(NQ):
                        pqk = psmm.tile([128, 512], F32, tag="mm")
                        for k in range(KC):
                            nc.tensor.matmul(
                                pqk[:],
                                wqk[:, half * KC * 128 + k * 128:
                                       half * KC * 128 + (k + 1) * 128],
                                XT[:, k * T + nt4 * 512: k * T + (nt4 + 1) * 512],
                                start=(k == 0), stop=(k == KC - 1))
                        nc.vector.tensor_scalar_add(
                            qkt[:, half * T + nt4 * 512: half * T + (nt4 + 1) * 512],
                            pqk[:],
                            bqk_sb[:, half * 4 + g: half * 4 + g + 1])

                # attention for heads (2g, 2g+1)
                for qt in (range(NQ) if 'ATT' in phases else ()):
                    psO0 = psO.tile([65, 512], F32, tag="o0")
                    psO1 = psO.tile([65, 512], F32, tag="o1")
                    psOh = [psO0, psO1]
                    jlast = 4 * qt + 3
                    # q-restriction per diagonal delta: computed q-range
                    # [qoff, 512) keeps fp32r matmuls at N>=256; causally-dead
                    # region is skipped, only the [128|256]-wide triangle
                    # blocks get an affine_select.
                    QOFF = (0, 128, 256, 256)
                    for s in range(2 * qt + 2):
                        diag = s >= 2 * qt
                        for hi in (0, 1):
                            psSt = psS.tile([128, 1024], F32, tag="psS")
                            for dd in (0, 1):
                                j = 2 * s + dd
                                if diag:
                                    qoff = QOFF[j - 4 * qt]
                                else:
                                    qoff = 0
                                nc.tensor.matmul(
                                    psSt[:, dd * 512 + qoff:(dd + 1) * 512],
                                    qkt[64 * hi:64 * hi + 64,
                                        T + j * 128: T + (j + 1) * 128],
                                    qkt[64 * hi:64 * hi + 64,
                                        qt * 512 + qoff:(qt + 1) * 512],
                                    start=True, stop=True,
                                    tile_position=(64 * hi, 0))
                            ptile = ptp.tile([128, 1024], F32R, tag=f"pt{hi}")
                            if diag and s == 2 * qt + 1:
                                # deltas 2,3: only cols [256:512] per dd computed
                                nc.scalar.activation(
                                    ptile[:, 256:512], psSt[:, 256:512],
                                    EXP, scale=0.125)
                                nc.scalar.activation(
                                    ptile[:, 768:1024], psSt[:, 768:1024],
                                    EXP, scale=0.125)
                            else:
                                nc.scalar.activation(ptile[:], psSt[:], EXP, scale=0.125)
                            if diag:
                                for dd in (0, 1):
                                    delta = 2 * (s - 2 * qt) + dd
                                    qoff = QOFF[delta]
                                    if delta < 3:
                                        # triangle block: cols [128*delta,128*delta+128)
                                        # keep where (qq rel block) - kk >= 0
                                        sl = slice(dd * 512 + 128 * delta,
                                                   dd * 512 + 128 * delta + 128)
                                        nc.gpsimd.affine_select(
                                            out=ptile[:, sl], in_=ptile[:, sl],
                                            compare_op=ISGE, fill=0.0, base=0,
                                            pattern=[[1, 128]],
                                            channel_multiplier=-1)
                                    else:
                                        # delta 3: computed cols [256:512); dead zone
                                        # [256:384) plus triangle [384:512):
                                        # keep where qq-384-kk >= 0 (rel: rel-128-kk)
                                        sl = slice(dd * 512 + 256, (dd + 1) * 512)
                                        nc.gpsimd.affine_select(
                                            out=ptile[:, sl], in_=ptile[:, sl],
                                            compare_op=ISGE, fill=0.0, base=-128,
                                            pattern=[[1, 256]],
                                            channel_multiplier=-1)
                            h = 2 * g + hi
                            for dd in (0, 1):
                                j = 2 * s + dd
                                qoff = QOFF[j - 4 * qt] if diag else 0
                                nc.tensor.matmul(
                                    psOh[hi][0:65, qoff:512],
                                    VA[:, j * VW + h * 65: j * VW + (h + 1) * 65],
                                    ptile[:, dd * 512 + qoff:(dd + 1) * 512],
                                    start=(j == 0), stop=(j == jlast))
                    # normalize + store OT
                    r = rscp.tile([128, 1024], F32R, tag="r")
                    with nc.allow_low_precision(reason="softmax reciprocal"):
                        for hi in (0, 1):
                            nc.vector.reciprocal(
                                r[64:65, hi * 512:(hi + 1) * 512],
                                psOh[hi][64:65, :])
                    for hi in (0, 1):
                        bc = psmm.tile([64, 512], F32, tag="mm")
                        nc.tensor.matmul(
                            bc[:], ones_sb[64:65, 0:64],
                            r[64:65, hi * 512:(hi + 1) * 512],
                            start=True, stop=True)
                        bc_sb = rscp.tile([64, 512], F32, tag="bcsb")
                        nc.vector.tensor_copy(bc_sb[:], bc[:])
                        if hi == 0:
                            nc.vector.tensor_mul(
                                OT[0:64, g * T + qt * 512: g * T + (qt + 1) * 512],
                                psOh[0][0:64, :], bc_sb[:])
                        else:
                            otmp = rscp.tile([64, 512], F32R, tag="otmp")
                            nc.vector.tensor_mul(otmp[:], psOh[1][0:64, :], bc_sb[:])
                            nc.sync.dma_start(
                                out=OT[64:128, g * T + qt * 512: g * T + (qt + 1) * 512],
                                in_=otmp[:])

        # ---------- phase D: projection ----------
        if 'D' in phases:
          with (
            tc.tile_pool(name="wp", bufs=1) as wpp,
            tc.tile_pool(name="stage", bufs=3) as stagep,
          ):
            wp = wpp.tile([128, 4 * C], F32R)
            nc.sync.dma_start(
                out=wp[:].rearrange("p (g m) -> p g m", g=4),
                in_=wproj_d[:].rearrange("(g p) m -> p g m", p=128).bitcast(F32R),
            )
            for it in range(NT):
                stage = stagep.tile([128, C], F32, tag="stg")
                for n in (0, 1):
                    pp = psmm.tile([128, 512], F32, tag="mm")
                    for g in range(4):
                        nc.tensor.matmul(
                            pp[:],
                            OT[:, g * T + it * 128: g * T + (it + 1) * 128],
                            wp[:, g * C + n * 512: g * C + (n + 1) * 512],
                            start=(g == 0), stop=(g == 3))
                    nc.scalar.copy(stage[:, n * 512:(n + 1) * 512], pp[:])
                nc.sync.dma_start(
                    out=out_d[it * 128:(it + 1) * 128, :], in_=stage[:])

    nc.compile()
    return nc


def _in_maps(x, W_attn, b_attn, W_proj, b_proj):
    ones64 = np.ones((128, 128), np.float32)

    in_maps = []
    for core in range(N_CORES):
        b = core // 2
        hg = core % 2
        sl = slice(hg * 512, (hg + 1) * 512)
        w_qkv = np.concatenate(
            [W_attn[:, 0:1024][:, sl], W_attn[:, 1024:2048][:, sl],
             W_attn[:, 2048:3072][:, sl]], axis=1)
        bq = b_attn[0:1024][sl]
        bk = b_attn[1024:2048][sl]
        bv = b_attn[2048:3072][sl]
        # b_qk [128, 8]: col half*4+g holds bias for W cols (half,g) chunk
        b_qk = np.stack(
            [bq[g * 128:(g + 1) * 128] for g in range(4)]
            + [bk[g * 128:(g + 1) * 128] for g in range(4)], axis=1)
        b_v = np.broadcast_to(bv, (128, 512)).copy()
        in_maps.append({
            "x": np.ascontiguousarray(x[b]),
            "w_qkv": np.ascontiguousarray(w_qkv),
            "w_proj": np.ascontiguousarray(W_proj[sl, :]),
            "b_qk": np.ascontiguousarray(b_qk.astype(np.float32)),
            "b_v": b_v.astype(np.float32),
            "ones64": ones64,
        })
    return in_maps


def kernel(x, W_attn, b_attn, W_proj, b_proj, _trace=False):
    from concourse.bass_utils import run_bass_kernel_spmd

    x = np.asarray(x, dtype=np.float32)
    W_attn = np.asarray(W_attn, dtype=np.float32)
    b_attn = np.asarray(b_attn, dtype=np.float32)
    W_proj = np.asarray(W_proj, dtype=np.float32)
    b_proj = np.asarray(b_proj, dtype=np.float32)

    if "nc" not in _CACHE:
        _CACHE["nc"] = _build()
    nc = _CACHE["nc"]

    in_maps = _in_maps(x, W_attn, b_attn, W_proj, b_proj)
    res = run_bass_kernel_spmd(nc, in_maps, list(range(N_CORES)), trace=_trace)
    B = x.shape[0]
    out = np.empty((B, T, C), np.float32)
    for b in range(B):
        out[b] = res.results[2 * b]["out"] + res.results[2 * b + 1]["out"] + b_proj
    if _trace:
        _CACHE["last_result"] = res
    return out
